# revision 54
# baseline (speedup 1.0000x reference)
"""Trainium2 Bass kernel for an AttentionBlock (GroupNorm + 8-head attention
over 32x32 spatial + proj + residual), data-parallel over batch on 8 cores.

Per batch element (x: [C=512, n=1024]):
  h   = GroupNorm32(x) * scale + bias          (h stored fp8, channel-packed)
  q,k = Wq h, Wk h  (bf16; k-bias provably drops out of softmax; q-bias kept)
  v^T = h^T Wv^T    (fp8, n-major, DoubleRow-packed, + ones row for Z)
  S^T = k^T q / 8   per head (bf16 matmuls, two heads packed in PE quadrants)
  e   = exp(S/8 - 2.5)  -> fp8   [ACT engine big chunks + DVE Schraudolph share]
  o   = v e (fp8 DoubleRow), Z from the ones row; att = o / Z
  y   = x + Wp att + pb_eff      (pb_eff = pb + Wp @ vb, folded host-side)

All convs run fp8 DoubleRow with channel-pair packing (c = 256t + 128j + p).
PSUM is managed as one 8-bank arena of four [128,1024] half-slots.
"""

import numpy as np

import concourse.bacc as bacc
import concourse.bass as bass
import concourse.tile as tile
from concourse import mybir
from concourse.bass_utils import run_bass_kernel_spmd

F32 = mybir.dt.float32
I32 = mybir.dt.int32
BF16 = mybir.dt.bfloat16
FP8 = mybir.dt.float8e4
AF = mybir.ActivationFunctionType
OP = mybir.AluOpType

C = 512
NH = 8
D = 64
N = 1024
GROUPS = 32
GS = C // GROUPS
EPS = 1e-5
B_PER_CORE = 2
N_CORES = 8

CT = 4    # channel tiles of 128
NT = 8    # n tiles of 128
VG = 66   # vT per-head group stride (64 data + 1 ones + 1 pad)

LOG2E = 1.4426950408889634
EXP_SHIFT = -2.5          # softmax-invariant logit shift keeps e in fp8 range
# Schraudolph exp in bf16 bit domain: int16 bits = round(S*A16 + B16) is the
# bf16 pattern of ~exp(0.125*S + EXP_SHIFT) (3% approx err; softmax
# normalization cancels the common-mode part). A@V reads the bits via bitcast.
SCHRA_A16 = float((1 << 7) * LOG2E * 0.125)
SCHRA_B16 = float((1 << 7) * (127.0 - 0.0450466 + EXP_SHIFT * LOG2E))

I16 = mybir.dt.int16


def build_nc(has_qb, has_pb, dump=False):
    nc = bacc.Bacc()

    x_ext = nc.declare_dram_parameter("x", [B_PER_CORE, 128, CT, N], F32, isOutput=False)
    dbg_ext = None
    if dump:
        dbg_ext = nc.declare_dram_parameter("dbg", [2, NH, N], F32, isOutput=True)
    w_ext = {}
    for nm in ("q", "k", "v", "p"):
        w_ext[nm] = nc.declare_dram_parameter(f"{nm}w8", [128, 2, 2, C], FP8, isOutput=False)
    # packed per-channel vectors: [128, 4, CT] = (norm_scale, norm_bias, qb, pb_eff)
    vecs_ext = nc.declare_dram_parameter("vecs", [128, 4, CT], F32, isOutput=False)
    selr_ext = nc.declare_dram_parameter("selr", [128, CT, GROUPS], BF16, isOutput=False)
    sele_ext = nc.declare_dram_parameter("sele", [GROUPS, CT, 128], BF16, isOutput=False)
    out_ext = nc.declare_dram_parameter("out", [B_PER_CORE, 128, CT, N], F32, isOutput=True)

    zdram = nc.dram_tensor("zscratch", [B_PER_CORE, NH, N], F32)
    zrdram = nc.dram_tensor("zrscratch", [B_PER_CORE, NH, N], F32)

    with tile.TileContext(nc) as tc:
        with (
            tc.tile_pool(name="const", bufs=1) as const,
            tc.tile_pool(name="work", bufs=2) as work,
            tc.tile_pool(name="epool", bufs=7) as epool,
            tc.tile_pool(name="small", bufs=2) as small,
            tc.tile_pool(name="psum", bufs=1, space="PSUM") as psum,
        ):
            # ---- PSUM arena: four [128, 1024] half-slots (2 banks each) ----
            arena = psum.tile([128, 4, N], F32, name="arena")
            slot_ctr = [0]

            def take_slots(k):
                s = []
                for _ in range(k):
                    s.append(slot_ctr[0] % 4)
                    slot_ctr[0] += 1
                return s

            def span2(p_lo, n_p, s_first, s_second, c_lo=0, n_c=N):
                """[n_p, 2, n_c] AP over two half-slots in the given order."""
                a0 = arena[p_lo : p_lo + n_p, s_first, c_lo : c_lo + n_c]
                delta = (s_second - s_first) * N
                return bass.AP(
                    tensor=a0.tensor,
                    offset=a0.offset,
                    ap=[list(a0.ap[0]), [delta, 2], list(a0.ap[-1])],
                )

            # ---- persistent constants -----------------------------------
            w_sb = {}

            def load_weights(names):
                for nm in names:
                    w_sb[nm] = const.tile([128, 2, 2, C], FP8, name=f"w_{nm}")
                    nc.sync.dma_start(out=w_sb[nm], in_=w_ext[nm].ap())

            vecs_sb = const.tile([128, 4, CT], F32)
            nc.sync.dma_start(out=vecs_sb, in_=vecs_ext.ap())
            nsc_sb = vecs_sb[:, 0, :]
            nbi_sb = vecs_sb[:, 1, :]
            qb_sb = vecs_sb[:, 2, :]
            pb_sb = vecs_sb[:, 3, :]
            selr_sb = const.tile([128, CT, GROUPS], BF16)
            nc.sync.dma_start(out=selr_sb, in_=selr_ext.ap())
            sele_sb = const.tile([GROUPS, CT, 128], BF16)
            nc.sync.dma_start(out=sele_sb, in_=sele_ext.ap())
            eps_t = const.tile([GROUPS, 1], F32)
            nc.vector.memset(eps_t, EPS)
            esh_t = const.tile([128, 1], F32)
            nc.vector.memset(esh_t, EXP_SHIFT)
            st = {0: {}, 1: {}}

            # ---- fill-unit queue (paired emission keeps slot alignment) --
            fillq = []

            def fill_pair():
                for _ in range(2):
                    if fillq:
                        fillq.pop(0)()

            # ---- x load --------------------------------------------------
            def emit_x_load(b):
                x_sb = work.tile([128, CT, N], F32, tag="x", name=f"x{b}")
                st[b]["x"] = x_sb
                for ct in range(CT):
                    nc.sync.dma_start(out=x_sb[:, ct, :], in_=x_ext.ap()[b][:, ct, :])

            # ---- GroupNorm stats -> csr/cb2 ------------------------------
            def emit_stats(b):
                x_sb = st[b]["x"]
                cstats = small.tile([128, CT, 2, 6], F32, tag="cstats")
                for ct in range(CT):
                    for sg in range(2):
                        nc.vector.bn_stats(
                            out=cstats[:, ct, sg, :],
                            in_=x_sb[:, ct, sg * 512 : (sg + 1) * 512],
                        )
                # per-(channel, chunk): a = mean_e+mean_o, bvar = cnt*var sums,
                # c2 = mean_e^2+mean_o^2; group-reduce via selector matmul.
                prep = small.tile([128, CT, 2, 3], F32, tag="prep")
                nc.vector.tensor_add(
                    out=prep[:, :, :, 0], in0=cstats[:, :, :, 1], in1=cstats[:, :, :, 4]
                )
                nc.vector.tensor_add(
                    out=prep[:, :, :, 1], in0=cstats[:, :, :, 2], in1=cstats[:, :, :, 5]
                )
                nc.vector.scalar_tensor_tensor(
                    out=cstats[:, :, :, 0], in0=cstats[:, :, :, 1], scalar=0.0,
                    in1=cstats[:, :, :, 1], op0=OP.add, op1=OP.mult,
                )
                nc.vector.scalar_tensor_tensor(
                    out=cstats[:, :, :, 3], in0=cstats[:, :, :, 4], scalar=0.0,
                    in1=cstats[:, :, :, 4], op0=OP.add, op1=OP.mult,
                )
                nc.vector.tensor_add(
                    out=prep[:, :, :, 2], in0=cstats[:, :, :, 0], in1=cstats[:, :, :, 3]
                )
                cb16 = small.tile([128, CT, 2, 3], BF16, tag="cb16")
                nc.vector.tensor_copy(out=cb16, in_=prep)
                ss = take_slots(2)
                gps = arena[0:GROUPS, ss[0], 0:6]
                for ct in range(CT):
                    nc.tensor.matmul(
                        out=gps,
                        lhsT=selr_sb[:, ct, :],
                        rhs=cb16[:, ct, :, :].rearrange("p s f -> p (s f)"),
                        start=(ct == 0),
                        stop=(ct == CT - 1),
                    )
                gsb = small.tile([GROUPS, 6], F32, tag="gsb")
                nc.vector.tensor_copy(out=gsb, in_=gps)
                gmv = small.tile([GROUPS, 4], F32, tag="gmv")
                nc.vector.tensor_add(out=gmv[:, 0:3], in0=gsb[:, 0:3], in1=gsb[:, 3:6])
                nc.vector.scalar_tensor_tensor(
                    out=gmv[:, 1:2], in0=gmv[:, 1:2], scalar=1.0 / 256.0,
                    in1=gmv[:, 2:3], op0=OP.mult, op1=OP.add,
                )
                nc.vector.scalar_tensor_tensor(
                    out=gmv[:, 3:4], in0=gmv[:, 0:1], scalar=0.0,
                    in1=gmv[:, 0:1], op0=OP.add, op1=OP.mult,
                )
                nc.vector.tensor_sub(out=gmv[:, 1:2], in0=gmv[:, 1:2], in1=gmv[:, 3:4])
                # rstd = exp(-0.5 * ln(var + eps)); Ln/Exp share one ACT table set
                lnv = small.tile([GROUPS, 1], F32, tag="lnv")
                nc.scalar.activation(out=lnv, in_=gmv[:, 1:2], func=AF.Ln, bias=eps_t)
                nc.scalar.activation(out=gmv[:, 1:2], in_=lnv, func=AF.Exp, scale=-0.5)
                gm16 = small.tile([GROUPS, 2], BF16, tag="gm16")
                nc.vector.tensor_copy(out=gm16, in_=gmv[:, 0:2])
                cps = arena[:, ss[1], 0:8]
                for ct in range(CT):
                    nc.tensor.matmul(
                        out=cps[:, ct * 2 : ct * 2 + 2],
                        lhsT=sele_sb[:, ct, :],
                        rhs=gm16,
                        start=True,
                        stop=True,
                    )
                cmv = cps.rearrange("p (ct s) -> p ct s", s=2)
                csr = small.tile([128, CT], F32, tag="csr", name=f"csr{b}")
                nc.vector.tensor_mul(out=csr, in0=cmv[:, :, 1], in1=nsc_sb)
                cb2 = small.tile([128, CT], F32, tag="cb2", name=f"cb2{b}")
                nc.vector.tensor_mul(out=cb2, in0=cmv[:, :, 0], in1=csr)
                nc.vector.tensor_sub(out=cb2, in0=nbi_sb, in1=cb2)
                st[b]["csr"] = csr
                st[b]["cb2"] = cb2

            # ---- h apply (gpsimd): x -> h fp8 channel-packed -------------
            def emit_h(b):
                h8 = work.tile([128, 2, 2, N], FP8, tag="h8", name=f"h8_{b}")
                st[b]["h8"] = h8
                csr, cb2 = st[b]["csr"], st[b]["cb2"]
                x_sb = st[b]["x"]
                for t in range(2):
                    for j in range(2):
                        ct = 2 * t + j
                        nc.gpsimd.tensor_scalar(
                            out=h8[:, t, j, :], in0=x_sb[:, ct, :],
                            scalar1=csr[:, ct : ct + 1], scalar2=cb2[:, ct : ct + 1],
                            op0=OP.mult, op1=OP.add,
                        )

            # ---- convs (fp8 DoubleRow) -----------------------------------
            def prep_conv(b):
                # q/k staged fp8 conv-layout, then DMA-reshuffled into the
                # d-packed quad layout [32*hq + i, hg, j, n] (d = 32j + i)
                qt_sb = work.tile([128, CT, N], FP8, tag="qt", name=f"qt{b}")
                kt_sb = work.tile([128, CT, N], FP8, tag="kt", name=f"kt{b}")
                q8_sb = work.tile([128, 2, 2, N], FP8, tag="q8", name=f"q8_{b}")
                k8_sb = work.tile([128, 2, 2, N], FP8, tag="k8", name=f"k8_{b}")
                vt_sb = work.tile([128, NT // 2, 2, NH, VG], FP8, tag="vt", name=f"vt{b}")
                st[b].update(
                    {"qt": qt_sb, "kt": kt_sb, "q8": q8_sb, "k8": k8_sb, "vt": vt_sb}
                )
                nc.vector.memset(vt_sb[:, :, :, :, D : D + 1], 1.0)

            def conv_qk_unit(b, nm, ct):
                def emit():
                    h8 = st[b]["h8"]
                    tmp = st[b][nm + "t"]
                    packed = st[b][nm + "8"]
                    s = take_slots(1)[0]
                    for t in range(2):
                        for ch in range(2):
                            nc.tensor.matmul(
                                out=arena[:, s, ch * 512 : (ch + 1) * 512],
                                lhsT=w_sb[nm][:, t, :, ct * 128 : (ct + 1) * 128],
                                rhs=h8[:, t, :, ch * 512 : (ch + 1) * 512],
                                start=(t == 0),
                                stop=(t == 1),
                                perf_mode=mybir.MatmulPerfMode.DoubleRow,
                            )
                    src = arena[:, s, :]
                    if nm == "q" and has_qb:
                        if b == 0:
                            nc.scalar.activation(
                                out=tmp[:, ct, :], in_=src, func=AF.Identity,
                                bias=qb_sb[:, ct : ct + 1],
                            )
                        else:
                            nc.vector.tensor_scalar(
                                out=tmp[:, ct, :], in0=src,
                                scalar1=qb_sb[:, ct : ct + 1], scalar2=None, op0=OP.add,
                            )
                    else:
                        if b == 0:
                            nc.scalar.copy(out=tmp[:, ct, :], in_=src)
                        else:
                            nc.vector.tensor_copy(out=tmp[:, ct, :], in_=src)
                    # partition reshuffle into the quad-S layout
                    for hh in range(2):
                        h_ = 2 * ct + hh
                        hq, hg = h_ % 4, h_ // 4
                        for j in range(2):
                            nc.gpsimd.dma_start(
                                out=packed[32 * hq : 32 * hq + 32, hg, j, :],
                                in_=tmp[64 * hh + 32 * j : 64 * hh + 32 * j + 32, ct, :],
                            )
                return emit

            def conv_v_unit(b, ntp):
                def emit():
                    h8 = st[b]["h8"]
                    vt_sb = st[b]["vt"]
                    s = take_slots(1)[0]
                    for k in range(2):
                        nt = 2 * ntp + k
                        for t in range(2):
                            nc.tensor.matmul(
                                out=arena[:, s, k * 512 : (k + 1) * 512],
                                lhsT=h8[:, t, :, nt * 128 : (nt + 1) * 128],
                                rhs=w_sb["v"][:, t, :, :],
                                start=(t == 0),
                                stop=(t == 1),
                                perf_mode=mybir.MatmulPerfMode.DoubleRow,
                            )
                    nc.vector.tensor_copy(
                        out=vt_sb[:, ntp, :, :, 0:D],
                        in_=arena[:, s, :].rearrange("p (k h d) -> p k h d", k=2, d=D),
                    )
                return emit

            # ---- attention -----------------------------------------------
            def prep_att(b):
                att8 = work.tile([128, 2, 2, N], FP8, tag="att8", name=f"att8_{b}")
                st[b].update({"attz": {}, "att8": att8, "e": {}, "eb": {}})

            def emit_S(b, hg, mt, ch):
                """S^T for 4 heads (quad row-packed fp8-DR) over one 512-col
                n-chunk; one [128, 2048] exp chunk across the 4 banks."""
                q8, k8 = st[b]["q8"], st[b]["k8"]
                if mt % 2 == 0 and ch == 0:
                    e4 = epool.tile(
                        [128, 2, 4, N], FP8, tag="e", name=f"e{b}_{hg}_{mt // 2}"
                    )
                    st[b]["e"][(hg, mt // 2)] = e4
                e4 = st[b]["e"][(hg, mt // 2)]
                ss = sorted(take_slots(2))
                for hq in range(4):
                    nc.tensor.matmul(
                        out=arena[:, ss[hq // 2], (hq % 2) * 512 : (hq % 2) * 512 + 512],
                        lhsT=k8[32 * hq : 32 * hq + 32, hg, :, mt * 128 : (mt + 1) * 128],
                        rhs=q8[32 * hq : 32 * hq + 32, hg, :, ch * 512 : (ch + 1) * 512],
                        start=True,
                        stop=True,
                        perf_mode=mybir.MatmulPerfMode.DoubleRow,
                        tile_position=(32 * hq, 0),
                    )
                src = bass.AP(
                    tensor=arena[:, ss[0], :].tensor,
                    offset=arena[:, ss[0], 0:512].offset,
                    ap=[
                        list(arena[:, ss[0], :].ap[0]),
                        [(ss[1] - ss[0]) * N, 2],
                        [512, 2],
                        [1, 512],
                    ],
                )
                e_dst = e4[:, mt % 2, :, ch * 512 : (ch + 1) * 512].rearrange(
                    "p (a b) n -> p a b n", a=2
                )
                nc.scalar.activation(
                    out=e_dst, in_=src, func=AF.Exp, scale=0.125, bias=esh_t,
                )

            def emit_AV_unit(b, h_, ch):
                """one A@V accumulation chain -> [65, 512] in one bank"""
                vt_sb = st[b]["vt"]
                if h_ not in st[b]["attz"]:
                    st[b]["attz"][h_] = work.tile(
                        [D + 1, N], F32, tag="attz", bufs=6, name=f"attz{b}_{h_}"
                    )
                hg, hq = h_ // 4, h_ % 4
                s = take_slots(1)[0]
                st[b].setdefault("avs", {})[(h_, ch)] = s
                pso = arena[0 : D + 1, s, 0:512]
                for mtp in range(NT // 2):
                    nc.tensor.matmul(
                        out=pso,
                        lhsT=vt_sb[:, mtp, :, h_, 0 : D + 1],
                        rhs=st[b]["e"][(hg, mtp)][:, :, hq, ch * 512 : (ch + 1) * 512],
                        start=(mtp == 0),
                        stop=(mtp == NT // 2 - 1),
                        perf_mode=mybir.MatmulPerfMode.DoubleRow,
                    )

            def emit_AV_copy(b, h_):
                """drain both ch banks of one head into attz (Z row rides)"""
                attz = st[b]["attz"][h_]
                s0 = st[b]["avs"][(h_, 0)]
                s1 = st[b]["avs"][(h_, 1)]
                if s1 > s0:
                    nc.vector.tensor_copy(
                        out=attz.rearrange("p (c n) -> p c n", c=2),
                        in_=span2(0, D + 1, s0, s1, 0, 512),
                    )
                else:
                    for ci, s in ((0, s0), (1, s1)):
                        nc.vector.tensor_copy(
                            out=attz[:, ci * 512 : (ci + 1) * 512],
                            in_=arena[0 : D + 1, s, 0:512],
                        )

            def emit_zchain(b, hp):
                for hi in range(2):
                    nc.gpsimd.dma_start(
                        out=zdram.ap()[b][2 * hp + hi],
                        in_=st[b]["attz"][2 * hp + hi][D : D + 1, :],
                    )
                zr = small.tile([2, N], F32, tag="zr")
                nc.sync.dma_start(out=zr, in_=zdram.ap()[b][2 * hp : 2 * hp + 2])
                nc.vector.reciprocal_approx_fast(out=zr, in_=zr)
                nc.sync.dma_start(out=zrdram.ap()[b][2 * hp : 2 * hp + 2], in_=zr)

            def emit_norm(b, hp):
                att8 = st[b]["att8"]
                for hi in range(2):
                    h_ = 2 * hp + hi
                    attz = st[b]["attz"][h_]
                    rzb = small.tile([D, N], F32, tag="rzb", name=f"rzb{b}_{h_}")
                    r1 = zrdram.ap()[b][h_]  # [N]
                    src = bass.AP(
                        tensor=r1.tensor,
                        offset=r1.offset,
                        ap=[[0, D], list(r1.ap[0])],
                    )
                    nc.sync.dma_start(out=rzb, in_=src)
                    # gpsimd cores are partition-hardwired: only the hi=0 mul
                    # keeps in/out on the same partitions, so hi=1 stays on DVE
                    eng = nc.gpsimd if hi == 0 else nc.vector
                    eng.tensor_mul(
                        out=att8[64 * hi : 64 * hi + 64, hp // 2, hp % 2, :],
                        in0=attz[0:D, :],
                        in1=rzb,
                    )

            # ---- proj + residual + store ---------------------------------
            def proj_unit(b, ct):
                def emit():
                    att8 = st[b]["att8"]
                    x_sb = st[b]["x"]
                    s = take_slots(1)[0]
                    for ch in range(2):
                        for t in range(2):
                            nc.tensor.matmul(
                                out=arena[:, s, ch * 512 : (ch + 1) * 512],
                                lhsT=w_sb["p"][:, t, :, ct * 128 : (ct + 1) * 128],
                                rhs=att8[:, t, :, ch * 512 : (ch + 1) * 512],
                                start=(t == 0),
                                stop=(t == 1),
                                perf_mode=mybir.MatmulPerfMode.DoubleRow,
                            )
                    nc.vector.scalar_tensor_tensor(
                        out=x_sb[:, ct, :], in0=arena[:, s, :],
                        scalar=pb_sb[:, ct : ct + 1] if has_pb else 0.0,
                        in1=x_sb[:, ct, :], op0=OP.add, op1=OP.add,
                    )
                    nc.sync.dma_start(out=out_ext.ap()[b][:, ct, :], in_=x_sb[:, ct, :])
                return emit

            # =========================== schedule =========================
            emit_x_load(0)
            load_weights(("q", "k"))
            emit_stats(0)
            load_weights(("v", "p"))
            emit_h(0)
            prep_conv(0)
            for ct in range(CT):
                conv_qk_unit(0, "q", ct)()
                conv_qk_unit(0, "k", ct)()
            for ntp in range(NT // 2):
                conv_v_unit(0, ntp)()
            emit_x_load(1)
            emit_stats(1)
            emit_h(1)
            prep_conv(1)
            for ct in range(CT):
                fillq.append(conv_qk_unit(1, "q", ct))
                fillq.append(conv_qk_unit(1, "k", ct))
            for ntp in range(NT // 2):
                fillq.append(conv_v_unit(1, ntp))
            prep_att(0)

            # A@V work for 4-head group P is spread across the NEXT group's
            # S/exp stream (16 u-slots), so the PE never lumps.
            def av_dispatch(pb, pg, u):
                h0 = 4 * pg
                if u < 8:
                    hq, ch = divmod(u, 2)
                    emit_AV_unit(pb, h0 + hq, ch)
                    if ch == 1:
                        # copy in the same u-slot: a pending A@V bank must
                        # never survive into the next S-unit's slot claims
                        emit_AV_copy(pb, h0 + hq)
                        if hq % 2 == 1:
                            emit_zchain(pb, (h0 + hq) // 2)
                elif u == 10:
                    emit_norm(pb, 2 * pg)
                elif u == 13:
                    emit_norm(pb, 2 * pg + 1)

            def att_stream(b, carry, fill_add=None):
                for hg in range(2):
                    if hg > 0:
                        pb, pg = b, hg - 1
                    else:
                        pb, pg = carry if carry is not None else (None, None)
                    for u in range(16):
                        mt, ch = divmod(u, 2)
                        if fill_add and (hg, u) in fill_add:
                            fillq.extend(fill_add[(hg, u)])
                        # AV work first: its pending-copy banks must not be
                        # claimed by this u-slot's S matmuls before the copy
                        # is emitted (the copy would then read S logits as Z)
                        if pb is not None:
                            av_dispatch(pb, pg, u)
                        emit_S(b, hg, mt, ch)
                        if u in (12, 14) and fillq:
                            fillq.pop(0)()
                        if pb is None and u % 2 == 1 and fillq:
                            fillq.pop(0)()

            att_stream(0, None)
            prep_att(1)
            att_stream(
                1,
                (0, 1),
                fill_add={(1, 0): [proj_unit(0, ct) for ct in range(CT)]},
            )
            # tail: last 4-head group of b1
            for hq in range(4):
                emit_AV_unit(1, 4 + hq, 0)
                emit_AV_unit(1, 4 + hq, 1)
                emit_AV_copy(1, 4 + hq)
                if hq % 2 == 1:
                    emit_zchain(1, (4 + hq) // 2)
                if fillq:
                    fillq.pop(0)()
            emit_norm(1, 2)
            emit_norm(1, 3)
            for ct in range(CT):
                proj_unit(1, ct)()
            if dump:
                nc.sync.dma_start(out=dbg_ext.ap()[0], in_=zdram.ap()[0])
                nc.sync.dma_start(out=dbg_ext.ap()[1], in_=zrdram.ap()[0])

    nc.compile()
    return nc


def kernel(x, norm_scale, norm_bias, q_w, q_b, k_w, k_b, v_w, v_b, proj_w, proj_b):
    import ml_dtypes

    fp8 = ml_dtypes.float8_e4m3fn
    bf16 = ml_dtypes.bfloat16

    x = np.asarray(x, dtype=np.float32)
    b, c, hh, ww = x.shape
    assert (b, c, hh * ww) == (16, C, N)
    xr = np.ascontiguousarray(x.reshape(b, CT, 128, hh * ww).transpose(0, 2, 1, 3))

    def _w8(w):
        wT = np.asarray(w, np.float32).T  # [Cin, Cout]
        return np.ascontiguousarray(
            wT.reshape(2, 2, 128, C).transpose(2, 0, 1, 3).astype(fp8)
        )

    pb_eff = np.asarray(proj_b, np.float32) + np.asarray(proj_w, np.float32) @ np.asarray(
        v_b, np.float32
    )
    vecs = np.stack(
        [
            np.asarray(v, np.float32).reshape(CT, 128).T
            for v in (norm_scale, norm_bias, q_b, pb_eff)
        ],
        axis=1,
    )  # [128, 4, CT]
    selr = np.zeros((128, CT, GROUPS), np.float32)
    sele = np.zeros((GROUPS, CT, 128), np.float32)
    for ct in range(CT):
        for p in range(128):
            g = ct * 8 + p // GS
            selr[p, ct, g] = 1.0 / 64.0
            sele[g, ct, p] = 1.0

    wts = {
        "qw8": _w8(q_w),
        "kw8": _w8(k_w),
        "vw8": _w8(v_w),
        "pw8": _w8(proj_w),
        "vecs": np.ascontiguousarray(vecs),
        "selr": np.ascontiguousarray(selr.astype(bf16)),
        "sele": np.ascontiguousarray(sele.astype(bf16)),
    }
    has_qb = bool(np.any(np.asarray(q_b)))
    has_pb = bool(np.any(pb_eff))

    nc = build_nc(has_qb, has_pb)
    in_maps = []
    for i in range(N_CORES):
        m = dict(wts)
        m["x"] = np.ascontiguousarray(xr[i * B_PER_CORE : (i + 1) * B_PER_CORE])
        in_maps.append(m)

    res = run_bass_kernel_spmd(nc, in_maps, core_ids=list(range(N_CORES)))
    kernel.last_result = res
    out = np.concatenate([res.results[i]["out"] for i in range(N_CORES)], axis=0)
    out = out.transpose(0, 2, 1, 3).reshape(b, c, hh, ww)
    return np.ascontiguousarray(out).astype(np.float32)
